# revision 1
# baseline (speedup 1.0000x reference)
"""Trainium2 Bass kernel for nn_DiscreteModel (GNN message passing).

Strategy: shard by node rows across 8 cores (512 rows each). All per-node
tensors are kept feature-major ([feature, node]) on-chip so the contraction
dim of every matmul sits on SBUF partitions. The host pre-transposes the
od_mat shard and all weights, folds the random-walk projection W_rw and the
1/8 mean into the layer-1 weight block, and pads HID 2112 -> 2176.

v2 changes vs the 196us baseline:
  fp8    : the od x W1 block (K=4096 of 4224) runs in fp8e4 DoubleRow mode
           (2 k-tiles per PE instruction, 2x bf16 rate). W1od is scaled x16
           on host (relu(16x)=16relu(x); 1/16 folded into W2). Emulated
           end-to-end rel err 1.2e-2 vs the 2e-2 gate.
  gather : one InstDMAGatherAnt (Q7 SWDGE ucode) replaces 32 indirect DMAs:
           ~2.4us issue + ~3us transfer vs ~49us serialized on Q0.
  tail   : sigmoid/tanh ACT table preloaded in the head (saves the 1.28us
           ACT_TABLE_LOAD on the GRU chain); GRU uses upd = zc*n + z*mem
           with zc/zm precomputed off the critical path; b2 folded into the
           GRU input bias on host; prediction matmuls in bf16 (was f32r,
           624->~250ns each); PSUM evacuation rotates vector/scalar/gpsimd;
           output staged + written as bf16 (host upcasts).
All DMAs use plain partition-major APs; every layout permutation happens on
the host (the Tile race tracker mis-handles partition-not-first dest APs).
"""

import numpy as np

import concourse.bass as bass
import concourse.bacc as bacc
import concourse.tile as tile
from concourse import mybir
from concourse.masks import make_identity

N = 4096        # nodes
MD = 64         # memory dim
MSG = 64        # message dim
WL = 8          # walk length
HID = 2112
HT = 17         # h-tiles (HID padded to 17*128 = 2176)
HIDP = HT * 128
NC = 8          # cores
R = N // NC     # rows (nodes) per core = 512
NT = R // 128   # node tiles per core = 4
F32 = mybir.dt.float32
F32R = mybir.dt.float32r
BF16 = mybir.dt.bfloat16
FP8 = mybir.dt.float8e4
I16 = mybir.dt.int16
WSCALE = 16.0   # W1 block scale so fp8 weights sit in e4m3 normal range
USE_DMA_GATHER = False   # one Q7 SWDGE gather vs 32 indirect DMAs
I32 = mybir.dt.int32

_PROG = None


def _build_program():
    nc = bacc.Bacc("TRN2", target_bir_lowering=False, debug=False, num_devices=NC)

    # ---- DRAM I/O (all pre-laid-out on host, partition-major) ----
    mem_d = nc.dram_tensor("mem", [N, MD], F32, kind="ExternalInput").ap()
    memT_d = nc.dram_tensor("memT", [MD, R], BF16, kind="ExternalInput").ap()
    memf_d = nc.dram_tensor("memf", [MD, R], F32R, kind="ExternalInput").ap()
    od_d = nc.dram_tensor("odv", [128, 32 * R], FP8, kind="ExternalInput").ap()
    if USE_DMA_GATHER:
        widx_d = nc.dram_tensor("widx", [128, NT * WL * 128 // 16], I16,
                                kind="ExternalInput").ap()
    else:
        widx32_d = nc.dram_tensor("widx", [128, NT * WL], I32,
                                  kind="ExternalInput").ap()
    w1h_d = nc.dram_tensor("w1h", [HT, 128, 32 * 128], FP8, kind="ExternalInput").ap()
    w1m_d = nc.dram_tensor("w1m", [128, HT * 128], BF16, kind="ExternalInput").ap()
    w2t_d = nc.dram_tensor("w2t", [128, HT * MSG], BF16, kind="ExternalInput").ap()
    wi_d = nc.dram_tensor("wi", [MSG, 3 * MD], F32R, kind="ExternalInput").ap()
    wh_d = nc.dram_tensor("wh", [MD, 3 * MD], F32R, kind="ExternalInput").ap()
    wp1_d = nc.dram_tensor("wp1", [MD, MD], F32R, kind="ExternalInput").ap()
    wp2_d = nc.dram_tensor("wp2", [MD, N], BF16, kind="ExternalInput").ap()
    bias_d = nc.dram_tensor("biases", [128, 64], F32, kind="ExternalInput").ap()
    out_d = nc.dram_tensor("outm", [32, 128, R], BF16, kind="ExternalOutput").ap()

    AF = mybir.ActivationFunctionType
    DR = mybir.MatmulPerfMode.DoubleRow

    with tile.TileContext(nc) as tc:
        with (
            tc.tile_pool(name="consts", bufs=1) as consts,
            tc.tile_pool(name="w1p", bufs=3) as w1p,
            tc.tile_pool(name="gp", bufs=2) as gp,
            tc.tile_pool(name="hp", bufs=5) as hp,
            tc.tile_pool(name="gates", bufs=1) as gates,
            tc.tile_pool(name="ostg", bufs=2) as ostg,
            tc.tile_pool(name="pmm", bufs=6, space="PSUM") as pmm,
            tc.tile_pool(name="pacc", bufs=1, space="PSUM") as pacc,
        ):
            # ---- walk indices first: the gather is the longest head chain
            if USE_DMA_GATHER:
                wk = consts.tile([128, NT * WL * 128 // 16], I16, tag="wk")
                nc.gpsimd.dma_start(out=wk[:], in_=widx_d[:])
            else:
                wk = consts.tile([128, NT * WL], I32, tag="wk")
                nc.gpsimd.dma_start(out=wk[:], in_=widx32_d[:])

            # head DMA order: h=0 consumes ALL 32 od k-tiles plus w1t[0], so
            # w1t[0] leads sync (before the od chunks) and the od back half +
            # w1t[1] lead scalar; biases/memT/memf are not needed until ~50us
            odres = consts.tile([128, 32 * R], FP8, tag="odres")
            w1t0 = w1p.tile([128, 32 * 128], FP8, tag="w1t")
            w1t1 = w1p.tile([128, 32 * 128], FP8, tag="w1t")
            w1ts = [w1t0, w1t1]  # w1t2 appended after its DMA below
            HKC = 16 * 128
            nc.sync.dma_start(out=w1ts[0][:, :HKC], in_=w1h_d[0][:, :HKC])
            nc.sync.dma_start(out=w1ts[0][:, HKC:], in_=w1h_d[0][:, HKC:])
            CH = 4 * R
            for c in range(4):
                nc.sync.dma_start(
                    out=odres[:, c * CH:(c + 1) * CH],
                    in_=od_d[:, c * CH:(c + 1) * CH],
                )
                nc.scalar.dma_start(
                    out=odres[:, (c + 4) * CH:(c + 5) * CH],
                    in_=od_d[:, (c + 4) * CH:(c + 5) * CH],
                )
            nc.scalar.dma_start(out=w1ts[1][:, :HKC], in_=w1h_d[1][:, :HKC])
            nc.scalar.dma_start(out=w1ts[1][:, HKC:], in_=w1h_d[1][:, HKC:])
            w1t2 = w1p.tile([128, 32 * 128], FP8, tag="w1t")
            nc.scalar.dma_start(out=w1t2[:], in_=w1h_d[2])
            w1ts.append(w1t2)

            # one Q7 SWDGE gather for all 4096 walk rows:
            # gare[p, (t*WL+j)*MD : +MD] = mem[walks[t*128+p, j]]
            gare = consts.tile([128, NT * WL * MD], F32, tag="gare")
            if USE_DMA_GATHER:
                # >=2048 idxs per instruction hangs the Q7 ucode on HW;
                # 4x1024 (one per node-tile) costs ~1.3us SWDGE each
                for t in range(NT):
                    nc.gpsimd.dma_gather(
                        gare[:, t * WL * MD:(t + 1) * WL * MD].rearrange(
                            "p (g d) -> p g d", g=WL),
                        mem_d[:], wk[:, t * WL * 8:(t + 1) * WL * 8],
                        WL * 128, WL * 128, MD,
                    )
            else:
                for t in range(NT):
                    for j in range(WL):
                        o = (t * WL + j) * MD
                        nc.gpsimd.indirect_dma_start(
                            out=gare[:, o:o + MD],
                            out_offset=None,
                            in_=mem_d[:],
                            in_offset=bass.IndirectOffsetOnAxis(
                                ap=wk[:, t * WL + j:t * WL + j + 1], axis=0),
                        )

            ident = consts.tile([128, 128], F32, tag="ident")
            make_identity(nc, ident[:])
            biasp = consts.tile([128, 64], F32, tag="biasp")
            nc.scalar.dma_start(out=biasp[:], in_=bias_d[:])

            # mixed rawT k-tile: [0:64] = memT shard, [64:128] = GsT (walk sums)
            mixed = consts.tile([128, R], BF16, tag="mixed")
            nc.scalar.dma_start(out=mixed[0:MD, :], in_=memT_d[:])
            memf = consts.tile([MD, R], F32R, tag="memf")
            nc.scalar.dma_start(out=memf[:], in_=memf_d[:])
            for c in range(2, 8):
                nc.scalar.dma_start(
                    out=odres[:, c * CH:(c + 1) * CH],
                    in_=od_d[:, c * CH:(c + 1) * CH],
                )

            # preload the sigmoid/tanh ACT table while the head is DMA-paced
            # (otherwise a 1.28us ACT_TABLE_LOAD lands on the GRU chain)
            warm = gates.tile([MD, 4], F32, tag="warm")
            nc.scalar.activation(warm[:, 0:2], biasp[0:MD, 0:2], AF.Sigmoid)
            nc.scalar.activation(warm[:, 2:4], biasp[0:MD, 0:2], AF.Tanh)

            def emit_mix_sums():
                # DVE tree-sums only (vector idles through the spill phase;
                # each sum self-gates on its node-tile's gather DMAs)
                for t in range(NT):
                    ga3 = gare[:, t * WL * MD:(t + 1) * WL * MD].rearrange(
                        "p (j d) -> p j d", j=WL)
                    m4 = gp.tile([128, 4 * MD], F32, tag="m4")
                    m43 = m4[:].rearrange("p (j d) -> p j d", j=4)
                    nc.vector.tensor_add(out=m43, in0=ga3[:, 0:4, :], in1=ga3[:, 4:8, :])
                    m2 = gp.tile([128, 2 * MD], F32, tag="m2")
                    m23 = m2[:].rearrange("p (j d) -> p j d", j=2)
                    nc.vector.tensor_add(out=m23, in0=m43[:, 0:2, :], in1=m43[:, 2:4, :])
                    m1t = gp.tile([128, MD], F32, tag=f"m1_{t}")
                    nc.vector.tensor_add(out=m1t[:], in0=m2[:, 0:MD],
                                         in1=m2[:, MD:2 * MD])
                    m1s[t] = m1t

            def emit_mix_transposes():
                # PE transposes + copies, emitted late enough that the PE
                # reaches them only after the gather+sums complete (~48us)
                for t in range(NT):
                    tr = pacc.tile([MD, 128], F32, tag="pred")
                    nc.tensor.transpose(out=tr[:], in_=m1s[t][:], identity=ident[:])
                    nc.vector.tensor_copy(
                        out=mixed[MD:128, t * 128:(t + 1) * 128], in_=tr[:])

            m1s = [None] * NT
            mixed_r = mixed[:]

            # ---- layer 1 (fp8 DoubleRow) + layer 2 accumulation ----
            # The gather data lands ~45us in while the DR stream alone would
            # reach h=8 by then, so groups 0..SPILL_H-1 spill their od-partial
            # to SBUF (freeing the PSUM bank) and replay once the mixed tile
            # exists; later groups run a 2-stage finalize (A: mixed matmul +
            # relu, B: L2 matmul one group later) so the PE never waits on
            # the relu chain.
            ps_msg = pacc.tile([MSG, R], F32, tag="msg")
            SPILL_H = 11
            odp = consts.tile([128, SPILL_H * R], F32, tag="odp")
            hids = {}
            l2n = [0]

            def stage_a(h, ps):
                nc.tensor.matmul(
                    out=ps[:], lhsT=w1m_sb[:, h * 128:(h + 1) * 128],
                    rhs=mixed_r, start=False, stop=True,
                )
                hid = hp.tile([128, R], BF16, tag="hid")
                nc.scalar.activation(hid[:], ps[:], AF.Relu, bias=biasp[:, h:h + 1])
                hids[h] = hid

            def stage_b(h):
                nc.tensor.matmul(
                    out=ps_msg[:], lhsT=w2t_sb[:, h * MSG:(h + 1) * MSG],
                    rhs=hids.pop(h)[:],
                    start=(l2n[0] == 0), stop=(l2n[0] == HT - 1),
                )
                l2n[0] += 1

            def cleanup_mm(h):
                ps = pmm.tile([128, R], F32, tag="mm")
                nc.tensor.matmul(
                    out=ps[:], lhsT=w1m_sb[:, h * 128:(h + 1) * 128],
                    rhs=mixed_r, start=True, stop=True,
                )
                pre = gp.tile([128, R], F32, tag="clt")
                nc.vector.tensor_add(out=pre[:], in0=ps[:],
                                     in1=odp[:, h * R:(h + 1) * R])
                hid = hp.tile([128, R], BF16, tag="hid")
                nc.scalar.activation(hid[:], pre[:], AF.Relu, bias=biasp[:, h:h + 1])
                hids[h] = hid

            aq = []  # groups awaiting stage A
            bq = []  # groups awaiting stage B
            cq = list(range(SPILL_H))   # spilled groups awaiting cleanup
            cb = []  # cleanup groups awaiting their L2
            for h in range(HT):
                if h == 2:
                    w1m_sb = consts.tile([128, HT * 128], BF16, tag="w1m")
                    nc.scalar.dma_start(out=w1m_sb[:], in_=w1m_d[:])
                    w2t_sb = consts.tile([128, HT * MSG], BF16, tag="w2t")
                    nc.scalar.dma_start(out=w2t_sb[:], in_=w2t_d[:])
                    wh_sb = consts.tile([MD, 3 * MD], F32R, tag="wh")
                    nc.scalar.dma_start(out=wh_sb[:], in_=wh_d[:])
                    emit_mix_sums()
                if h == 4:
                    wi_sb = consts.tile([MSG, 3 * MD], F32R, tag="wi")
                    nc.scalar.dma_start(out=wi_sb[:], in_=wi_d[:])
                    wp1_sb = consts.tile([MD, MD], F32R, tag="wp1")
                    nc.scalar.dma_start(out=wp1_sb[:], in_=wp1_d[:])
                    wp2_sb = consts.tile([MD, N], BF16, tag="wp2")
                    nc.scalar.dma_start(out=wp2_sb[:], in_=wp2_d[:])
                if h < 3:
                    w1t = w1ts[h]
                else:
                    w1t = w1p.tile([128, 32 * 128], FP8, tag="w1t")
                    eng = nc.scalar if h % 4 == 3 else nc.sync
                    eng.dma_start(out=w1t[:], in_=w1h_d[h])
                ps = pmm.tile([128, R], F32, tag="mm")
                for k in range(16):
                    nc.tensor.matmul(
                        out=ps[:],
                        lhsT=w1t[:, k * 256:(k + 1) * 256].rearrange(
                            "p (two m) -> p two m", two=2),
                        rhs=odres[:, k * 2 * R:(k + 1) * 2 * R].rearrange(
                            "p (two n) -> p two n", two=2),
                        start=(k == 0), stop=(k == 15 and h < SPILL_H),
                        perf_mode=DR,
                    )
                if h < SPILL_H:
                    # evacuate pre-activation od partial; scalar only (the
                    # vector stream is held by the gather-gated mix sums)
                    nc.scalar.activation(odp[:, h * R:(h + 1) * R], ps[:],
                                         AF.Identity)
                else:
                    aq.append((h, ps))
                if h == 3:
                    # h_n = memT @ Wh_n + bias depends only on memf; do it in
                    # the ramp where the PE has slack
                    ps_hn = pmm.tile([MD, R], F32, tag="mm")
                    nc.tensor.matmul(out=ps_hn[:], lhsT=wh_sb[:, 128:192],
                                     rhs=memf[:], start=True, stop=True)
                    hnb = gates.tile([MD, R], F32, tag="hnb")
                    nc.vector.tensor_scalar_add(out=hnb[:], in0=ps_hn[:],
                                                scalar1=biasp[0:MD, 20:21])
                if h == SPILL_H:
                    # PE reaches this point right as the gather+sums finish
                    emit_mix_transposes()
                if h >= SPILL_H + 1:
                    c = cq.pop(0)
                    cleanup_mm(c)
                    cb.append(c)
                if h >= SPILL_H + 2:
                    stage_b(cb.pop(0))
                if h >= SPILL_H + 3 and aq:
                    bq.append(aq[0][0])
                    stage_a(*aq.pop(0))
                if h >= SPILL_H + 4 and bq:
                    stage_b(bq.pop(0))
            # drain: interleave remaining A/B/cleanup so every relu has a
            # full matmul-group of slack before its L2
            while aq or bq or cq or cb:
                if aq:
                    bq.append(aq[0][0])
                    stage_a(*aq.pop(0))
                if cq:
                    c = cq.pop(0)
                    cleanup_mm(c)
                    cb.append(c)
                if bq:
                    stage_b(bq.pop(0))
                if cb:
                    stage_b(cb.pop(0))

            msg_sb = gates.tile([MSG, R], F32R, tag="msg_sb")
            nc.vector.tensor_copy(out=msg_sb[:], in_=ps_msg[:])
            msg_r = msg_sb[:]
            memT_r = memf[:]

            # ---- GRU + prediction, column-split so the serial
            #      ACT/DVE chain pipelines across halves. r and z go to
            #      separate partition-0 PSUM tiles: the BIR verifier rejects
            #      tensor ops whose SBUF operands start at different
            #      partitions, and the r chain can start after 2 matmuls.
            ps_r = pmm.tile([MD, R], F32, tag="mm")
            nc.tensor.matmul(out=ps_r[:], lhsT=wi_sb[:, 0:MD], rhs=msg_r,
                             start=True, stop=False)
            nc.tensor.matmul(out=ps_r[:], lhsT=wh_sb[:, 0:MD], rhs=memT_r,
                             start=False, stop=True)
            ps_z = pmm.tile([MD, R], F32, tag="mm")
            nc.tensor.matmul(out=ps_z[:], lhsT=wi_sb[:, MD:128], rhs=msg_r,
                             start=True, stop=False)
            nc.tensor.matmul(out=ps_z[:], lhsT=wh_sb[:, MD:128], rhs=memT_r,
                             start=False, stop=True)
            ps_in = pmm.tile([MD, R], F32, tag="mm")
            nc.tensor.matmul(out=ps_in[:], lhsT=wi_sb[:, 128:192], rhs=msg_r,
                             start=True, stop=True)
            r_t = gates.tile([MD, R], F32, tag="r_t")
            z_t = gates.tile([MD, R], F32, tag="z_t")
            rhn = gates.tile([MD, R], F32, tag="rhn")
            npre = gates.tile([MD, R], F32, tag="npre")
            n_t = gates.tile([MD, R], F32, tag="n_t")
            zc_t = gates.tile([MD, R], F32, tag="zc_t")
            zm_t = gates.tile([MD, R], F32, tag="zm_t")
            ncz = gates.tile([MD, R], F32, tag="ncz")
            upd = gates.tile([MD, R], F32R, tag="upd")
            ps_pred = pacc.tile([MD, R], F32, tag="pred")
            act = gates.tile([MD, R], BF16, tag="act")
            HR = R // 2
            for x in range(2):
                cs = slice(x * HR, (x + 1) * HR)
                nc.scalar.activation(r_t[:, cs], ps_r[:, cs], AF.Sigmoid,
                                     bias=biasp[0:MD, 17:18])
                nc.scalar.activation(z_t[:, cs], ps_z[:, cs], AF.Sigmoid,
                                     bias=biasp[0:MD, 18:19])
                # upd = (1-z)*n + z*mem = zc*n + zm; zc/zm run off-chain on
                # gpsimd while the r -> n tanh chain holds scalar+vector
                nc.gpsimd.tensor_scalar(out=zc_t[:, cs], in0=z_t[:, cs],
                                        scalar1=-1.0, scalar2=1.0,
                                        op0=mybir.AluOpType.mult,
                                        op1=mybir.AluOpType.add)
                nc.gpsimd.tensor_mul(out=zm_t[:, cs], in0=z_t[:, cs],
                                     in1=memf[:, cs].bitcast(F32))
                nc.vector.tensor_mul(out=rhn[:, cs], in0=r_t[:, cs], in1=hnb[:, cs])
                nc.vector.tensor_add(out=npre[:, cs], in0=ps_in[:, cs], in1=rhn[:, cs])
                nc.scalar.activation(n_t[:, cs], npre[:, cs], AF.Tanh,
                                     bias=biasp[0:MD, 19:20])
                nc.vector.tensor_mul(out=ncz[:, cs], in0=zc_t[:, cs], in1=n_t[:, cs])
                nc.vector.tensor_add(out=upd[:, cs], in0=ncz[:, cs], in1=zm_t[:, cs])
                nc.tensor.matmul(out=ps_pred[:, cs], lhsT=wp1_sb[:], rhs=upd[:, cs],
                                 start=True, stop=True)
                nc.scalar.activation(act[:, cs], ps_pred[:, cs], AF.Relu,
                                     bias=biasp[0:MD, 21:22])
            act_r = act[:]

            # ---- prediction m-loop: bf16 matmuls; PSUM evacuation rotates
            #      vector/scalar/gpsimd; output staged bf16, 1MB DMAs on sync
            GRP = 8
            for m in range(32):
                ps_o = pmm.tile([128, R], F32, tag="mm")
                nc.tensor.matmul(out=ps_o[:], lhsT=wp2_sb[:, m * 128:(m + 1) * 128],
                                 rhs=act_r, start=True, stop=True)
                if m % GRP == 0:
                    stage = ostg.tile([128, GRP * R], BF16, tag="stage")
                sl = stage[:, (m % GRP) * R:(m % GRP + 1) * R]
                # gpsimd can't read PSUM; the ~690-750ns PSUM read paces the
                # loop, so split evacuations evenly across scalar and vector
                if m % 2 == 1:
                    nc.vector.tensor_scalar_add(out=sl, in0=ps_o[:],
                                                scalar1=biasp[:, 22 + m:23 + m])
                else:
                    nc.scalar.activation(sl, ps_o[:], AF.Identity,
                                         bias=biasp[:, 22 + m:23 + m])
                if m % GRP == GRP - 1:
                    g = m // GRP
                    oeng = nc.sync if g % 2 == 0 else nc.gpsimd
                    oeng.dma_start(
                        out=out_d[g * GRP:(g + 1) * GRP].rearrange(
                            "g p n -> p g n"),
                        in_=stage[:].rearrange("p (g n) -> p g n", g=GRP))

    nc.compile()
    return nc


def _get_program():
    global _PROG
    if _PROG is None:
        _PROG = _build_program()
    return _PROG


def _host_prep(memory, od_mat, walks, W_rw, b_rw, W1, b1, W2, b2,
               gru_Wi, gru_bi, gru_Wh, gru_bh, Wp1, bp1, Wp2, bp2):
    import ml_dtypes
    f = np.float32
    bf = ml_dtypes.bfloat16
    e4 = ml_dtypes.float8_e4m3fn
    memory = np.ascontiguousarray(np.asarray(memory), dtype=f)
    od_mat = np.asarray(od_mat)
    walks = np.asarray(walks).astype(np.int32)
    W_rw = np.asarray(W_rw, dtype=f); b_rw = np.asarray(b_rw, dtype=f)
    W1 = np.asarray(W1, dtype=f); b1 = np.asarray(b1, dtype=f)
    W2 = np.asarray(W2, dtype=f); b2 = np.asarray(b2, dtype=f)
    gru_Wi = np.asarray(gru_Wi, dtype=f); gru_bi = np.asarray(gru_bi, dtype=f)
    gru_Wh = np.asarray(gru_Wh, dtype=f); gru_bh = np.asarray(gru_bh, dtype=f)
    Wp1 = np.asarray(Wp1, dtype=f); bp1 = np.asarray(bp1, dtype=f)
    Wp2 = np.asarray(Wp2, dtype=f); bp2 = np.asarray(bp2, dtype=f)

    # layer-1 weights, column-permuted to [od | dest | walk] with W_rw and the
    # 1/8 mean folded into the walk block; HID padded to 2176; whole block
    # scaled x16 so the fp8 od weights sit in e4m3 normal range (1/16 folded
    # into W2; exact since relu(16x)=16relu(x))
    W1od = W1[:, MD:MD + N]
    W1dest = W1[:, 0:MD]
    W1rw = W1[:, MD + N:]
    W1g = (W1rw @ W_rw) / np.float32(8.0)
    W1p = np.concatenate([W1od, W1dest, W1g], axis=1) * np.float32(WSCALE)
    W1pT = np.zeros((33 * 128, HIDP), dtype=f)
    W1pT[:, :HID] = W1p.T
    # w1h[h][p, k*128+c] = W1pT[k*128+p, h*128+c] for the 32 od k-tiles
    # (pairs of adjacent k-tiles feed one DoubleRow matmul);
    # the mixed k-tile (rows 4096:4224) is its own resident tensor w1m
    w1h = np.ascontiguousarray(
        W1pT[:32 * 128].reshape(32, 128, HT, 128)
        .transpose(2, 1, 0, 3).reshape(HT, 128, 32 * 128).astype(e4))
    w1m = np.ascontiguousarray(W1pT[32 * 128:].astype(bf))  # [128, 2176]

    b1p = np.zeros(HIDP, dtype=f)
    b1p[:HID] = (b1 + W1rw @ b_rw) * np.float32(WSCALE)

    W2tp = np.zeros((HIDP, MSG), dtype=f)
    W2tp[:HID] = W2.T / np.float32(WSCALE)
    # w2t[p, h*64+c] = W2tp[h*128+p, c]
    w2t = np.ascontiguousarray(
        W2tp.reshape(HT, 128, MSG).transpose(1, 0, 2).reshape(128, HT * MSG)
        .astype(bf))

    def pad128(v):
        o = np.zeros(128, dtype=f)
        o[:v.shape[0]] = v
        return o

    # b2 folded through the GRU input weights: gi = Wi@(msg'+b2)+bi
    gbi_f = gru_bi + gru_Wi @ b2

    # biases packed as [128 partitions, 64 columns]
    biases = np.zeros((64, 128), dtype=f)
    biases[0:HT] = b1p.reshape(HT, 128)
    grz = gbi_f[:128] + gru_bh[:128]
    biases[17] = pad128(grz[:64])      # r gate bias
    biases[18] = pad128(grz[64:])      # z gate bias
    biases[19] = pad128(gbi_f[128:])
    biases[20] = pad128(gru_bh[128:])
    biases[21] = pad128(bp1)
    biases[22:54] = bp2.reshape(32, 128)
    biases = np.ascontiguousarray(biases.T)                    # [128, 64]

    shared = {
        "mem": memory,
        "w1h": w1h,
        "w1m": w1m,
        "w2t": w2t,
        "wi": np.ascontiguousarray(gru_Wi.T),
        "wh": np.ascontiguousarray(gru_Wh.T),
        "wp1": np.ascontiguousarray(Wp1.T),
        "wp2": np.ascontiguousarray(Wp2.T.astype(bf)),
        "biases": biases,
    }
    in_maps = []
    for c in range(NC):
        sl = slice(c * R, (c + 1) * R)
        odc = np.asarray(od_mat[sl], dtype=f)
        # odv[p, k*R+n] = od[c*R+n, k*128+p]
        odv = np.ascontiguousarray(
            odc.T.reshape(32, 128, R).transpose(1, 0, 2).reshape(128, 32 * R)
            .astype(e4))
        # dma_gather wrap-16 idx layout: idx[(t*WL+j)*128+p] = walks[t*128+p, j]
        if USE_DMA_GATHER:
            wkc = walks[sl].reshape(NT, 128, WL).transpose(0, 2, 1).reshape(-1)
            # wrap-16 idx layout, replicated across the 8 Q7 core stripes
            widx = np.ascontiguousarray(np.tile(
                wkc.reshape(-1, 16).T.astype(np.int16), (8, 1)))  # [128, 256]
        else:
            # widx[p, t*WL+j] = walks[c*R + t*128 + p, j]
            widx = np.ascontiguousarray(
                walks[sl].reshape(NT, 128, WL).transpose(1, 0, 2)
                .reshape(128, NT * WL))
        memT = np.ascontiguousarray(memory[sl].T)
        in_maps.append(dict(
            shared,
            memT=np.ascontiguousarray(memT.astype(bf)),
            memf=memT,
            odv=odv,
            widx=widx,
        ))
    return in_maps


def _assemble(results):
    od = np.empty((N, N), dtype=np.float32)
    for c in range(NC):
        # outm[m, p, n] = od[c*R+n, m*128+p]
        od[c * R:(c + 1) * R, :] = (
            results[c]["outm"].astype(np.float32).transpose(2, 0, 1).reshape(R, N))
    return od


def _install_ntff_shim():
    """The agent image's antenv lacks axon_hooks, so trace=True dies on
    import. Recreate the module with the ctypes-based NTFF hook that
    trn_agent_boot would have registered."""
    import sys
    import types
    if "antenv.axon_hooks" in sys.modules:
        return
    from trn_agent_boot.trn_boot import _ntff_profile_via_ctypes
    hook = _ntff_profile_via_ctypes("/opt/axon/libaxon_pjrt.so")
    mod = types.ModuleType("antenv.axon_hooks")
    mod._hook = hook
    mod.get_axon_ntff_profile_hook = lambda: mod._hook
    mod.set_axon_ntff_profile_hook = lambda h: setattr(mod, "_hook", h)
    sys.modules["antenv.axon_hooks"] = mod


def run(inputs, trace=False):
    """Run on 8 NeuronCores; returns (od [N,N] f32, BassKernelResults)."""
    from concourse.bass_utils import run_bass_kernel_spmd
    if trace:
        try:
            _install_ntff_shim()
        except Exception as e:
            print(f"ntff shim failed ({e}); running without trace")
            trace = False
    nc = _get_program()
    in_maps = _host_prep(**inputs)
    res = run_bass_kernel_spmd(nc, in_maps, list(range(NC)), trace=trace)
    return _assemble(res.results), res


def kernel(**inputs):
    od, _ = run(inputs)
    return od

